# revision 10
# baseline (speedup 1.0000x reference)
"""Trainium2 Bass kernel for nn_NegSimHead (loss_fn).

Reference computation (N=8192, C=512):
  v = normalize(v_feat); t = normalize(t_feat); pv = normalize(p_v); pt = normalize(p_t)
  neg_sim = -0.5*mean(sum(pv*t,1)) - 0.5*mean(sum(pt*v,1))
  stats(x) = mean(std(x, axis=0, ddof=1)) for each normalized tensor
  s1 = v @ pt.T ; s2 = t @ pv.T
  retrieval(s): pos[i] = rank of s[i,i] in row i (descending) = #{j: s[i,j] > s[i,i]}
  out [13] = [neg_sim, stats(v), stats(t), stats(pv), stats(pt),
              r1,r5,r10,mr of s1, r1,r5,r10,mr of s2]

Strategy (8 cores):
  All four tensors are row-sharded: core k gets rows k*1024..(k+1)*1024 of
  v/t/pt/pv, transposed to [512,1024], cast to fp8 e4m3 on the host, and
  shipped as ONE [2048,1024] operand per core.  The per-call dispatch cost
  through the axon-tunneled PJRT path is ~1 network RTT (~70ms) plus costs
  that scale with bound-operand and fetched-output bytes, so the kernel
  binds only 2MB/core of fp8 shards (vs 36MB/core replicated f32 in the
  first version) and returns a single [1,16] row per core; the full p-hat
  matrices are reconstructed ON DEVICE with AllGathers over NeuronLink.

  Per core: normalize the local p shard (squares -> all-ones matmul
  partition-reduce -> rsqrt -> scale), round p-hat to fp8, and AllGather
  both phases' p-hat into shared DRAM [4096,1024] (block r = core r's
  rows).  Row-normalization of v/t scales whole rows of the similarity
  strip and cancels in the rank comparison, so raw fp8 x shards feed the
  matmuls directly.  The comparison threshold d~_i = x_i . p-hat_i is
  computed locally from the aligned shards (elementwise product ->
  all-ones matmul), which also yields the neg_sim loss term (scaled by
  1/||x_i||) -- no diagonal extraction and no per-core asymmetry (no roll,
  no partition-id addressing); exact-tie handling at the diagonal is
  unnecessary because the rank tolerance is ~82 absolute.  Counting is
  split between ScalarE (Sign with per-partition bias, fused accumulate)
  and VectorE (is_gt with per-partition scalar, fused accumulate).

  Finalization happens on device: per-core partials (stats sum/sumsq per
  feature, rank-threshold counts, loss) are packed into a [128,44] tile,
  AllReduced across the 8 cores, and every core computes the identical
  final [13] vector (per-feature ddof=1 std -> mean, r1/r5/r10/mean-rank,
  neg_sim).  The host just reads 13 floats from core 0.
"""
import time
import numpy as np
from contextlib import ExitStack

import concourse.bacc as bacc
import concourse.tile as tile
from concourse import mybir

F32 = mybir.dt.float32
F32R = mybir.dt.float32r
BF16 = mybir.dt.bfloat16
QDT = mybir.dt.float8e4
ALU = mybir.AluOpType
AX = mybir.AxisListType
AF = mybir.ActivationFunctionType

N = 8192          # batch
C = 512           # feature dim
NCORES = 8
S = N // NCORES   # rows per core = 1024
KC = C // 128     # contraction chunks = 4
MB = S // 128     # row strips per core = 8
NTILE = 512       # similarity column tile
NT = N // NTILE   # column tiles = 16
# column tiles 8..15 counted on ScalarE (Sign), 0..7 on VectorE (is_gt)
N_ACT = 8
C_ACT = N_ACT * NTILE

# comb (cross-core AllReduce payload) column layout
# stats partials st64: [tensor(4)][k(4)][s(2: sum,sq)][half(2)] -> packed to
# comb[0:32] as [s(2)][tensor(4)][k(4)]; metrics comb[32:40] =
# [phase(2)][r1cnt,r5cnt,r10cnt,possum]; loss comb[40:44] = [phase(2)][half(2)]
MCOL = 32
LCOL = 40
CCOLS = 44

_CACHE = {}
TIMES = {}


def _build_program():
    nc = bacc.Bacc("TRN2", target_bir_lowering=False, debug=False,
                   num_devices=NCORES)

    xin_d = nc.dram_tensor("xin", [4 * C, S], QDT, kind="ExternalInput").ap()
    xT_d = [xin_d[0:C, :], xin_d[C:2 * C, :]]
    pT_d = [xin_d[2 * C:3 * C, :], xin_d[3 * C:4 * C, :]]
    o_fin_d = nc.dram_tensor("o_fin", [1, 16], F32,
                             kind="ExternalOutput").ap()
    ident_d = nc.inline_tensor(np.eye(128, dtype=np.float32), name="ident").ap()

    with tile.TileContext(nc) as tc, ExitStack() as ctx:
        persist = ctx.enter_context(tc.tile_pool(name="persist", bufs=1))
        gpool = ctx.enter_context(tc.tile_pool(name="gpool", bufs=3))
        sq_pool = ctx.enter_context(tc.tile_pool(name="sq", bufs=2))
        scr_pool = ctx.enter_context(tc.tile_pool(name="scr", bufs=2))
        dram = ctx.enter_context(tc.tile_pool(name="dram", bufs=1,
                                              space="DRAM"))
        mm_psum = ctx.enter_context(tc.tile_pool(name="mmps", bufs=6,
                                                 space="PSUM"))
        nrm_psum = ctx.enter_context(tc.tile_pool(name="nrmps", bufs=2,
                                                  space="PSUM"))

        i_t = persist.tile([128, 128], F32, name="i_t")
        nc.sync.dma_start(out=i_t, in_=ident_d)
        ones_t = persist.tile([128, 128], F32, name="ones_t")
        nc.vector.memset(ones_t, 1.0)

        st64 = persist.tile([128, 64], F32, name="st64")
        comb = persist.tile([128, CCOLS], F32, name="comb")
        fin = persist.tile([128, 16], F32, name="fin")

        # bf16 input shards, resident for the whole kernel
        xT = [[persist.tile([128, S], QDT, name=f"xT{ph}_{k}")
               for k in range(KC)] for ph in range(2)]
        pT = [[persist.tile([128, S], QDT, name=f"pT{ph}_{k}")
               for k in range(KC)] for ph in range(2)]
        phat = [[persist.tile([128, S], QDT, name=f"phat{ph}_{k}")
                 for k in range(KC)] for ph in range(2)]

        invb_x = [persist.tile([128, S], F32, name=f"invb_x{ph2}")
                  for ph2 in range(2)]
        invb_p = [persist.tile([128, S], F32, name=f"invb_p{ph2}")
                  for ph2 in range(2)]
        dT = [persist.tile([128, MB], F32, name=f"dT{ph2}") for ph2 in range(2)]
        negdT = [persist.tile([128, MB], F32, name=f"negdT{ph2}")
                 for ph2 in range(2)]
        cnts = [persist.tile([128, MB, NT], F32, name=f"cnts{ph2}")
                for ph2 in range(2)]
        sgns = [persist.tile([128, MB, NT], F32, name=f"sgns{ph2}")
                for ph2 in range(2)]
        for ph in range(2):
            nc.vector.memset(cnts[ph], 0.0)
            nc.vector.memset(sgns[ph], 0.0)

        # DRAM bounce + gathered (Shared) buffers for the collectives
        cc_in = [dram.tile([C, S], QDT, name=f"cc_in{ph}") for ph in range(2)]
        gath = [dram.tile([NCORES * C, S], QDT, name=f"gath{ph}",
                          addr_space="Shared") for ph in range(2)]

        def sumsq_inv(src, dst, tag):
            # dst[:, j] = 1/sqrt(sum_c src[c, j]^2), partition-broadcast
            for h in range(2):
                hs = slice(h * 512, (h + 1) * 512)
                ps = nrm_psum.tile([128, 512], F32, name=f"ps_{tag}_{h}",
                                   tag="nrm")
                for k in range(KC):
                    sq = sq_pool.tile([128, 512], F32,
                                      name=f"sq_{tag}_{k}_{h}", tag="sq")
                    nc.scalar.square(sq, src[k][:, hs])
                    nc.tensor.matmul(ps, ones_t, sq,
                                     start=(k == 0), stop=(k == KC - 1))
                nc.vector.reciprocal(dst[:, hs], ps)
                nc.scalar.sqrt(dst[:, hs], dst[:, hs])

        def stage1(ph):
            # load p shard, normalize, round to fp8, bounce out, all-gather
            for k in range(KC):
                nc.sync.dma_start(out=pT[ph][k],
                                  in_=pT_d[ph][k * 128:(k + 1) * 128, :])
            sumsq_inv(pT[ph], invb_p[ph], f"p{ph}")
            for k in range(KC):
                nc.vector.tensor_mul(phat[ph][k], pT[ph][k], invb_p[ph])
                nc.sync.dma_start(out=cc_in[ph][k * 128:(k + 1) * 128, :],
                                  in_=phat[ph][k])
            nc.gpsimd.collective_compute(
                "AllGather", ALU.bypass,
                replica_groups=[list(range(NCORES))],
                ins=[cc_in[ph].opt()],
                outs=[gath[ph].opt()])

        def stage2(ph):
            # x norm, stats (x-hat and p-hat), threshold d~, loss
            sumsq_inv(xT[ph], invb_x[ph], f"x{ph}")
            pstat = 3 if ph == 0 else 2  # pt=3, pv=2 in stats layout
            for k in range(KC):
                xh = scr_pool.tile([128, S], F32, name=f"xh{ph}_{k}", tag="xh")
                nc.vector.tensor_mul(xh, xT[ph][k], invb_x[ph])
                for h in range(2):
                    hs = slice(h * 512, (h + 1) * 512)
                    col = ph * 16 + k * 4 + h          # x sums (s=0)
                    nc.vector.tensor_reduce(st64[:, col:col + 1], xh[:, hs],
                                            axis=AX.X, op=ALU.add)
                    xscr = scr_pool.tile([128, 512], F32,
                                         name=f"xs{ph}_{k}_{h}", tag="scr")
                    nc.scalar.activation(out=xscr, in_=xh[:, hs],
                                         func=AF.Square,
                                         accum_out=st64[:, col + 2:col + 3])
                    pcol = pstat * 16 + k * 4 + h
                    nc.vector.tensor_reduce(st64[:, pcol:pcol + 1],
                                            phat[ph][k][:, hs],
                                            axis=AX.X, op=ALU.add)
                    pscr = scr_pool.tile([128, 512], F32,
                                         name=f"psc{ph}_{k}_{h}", tag="scr")
                    nc.scalar.activation(out=pscr, in_=phat[ph][k][:, hs],
                                         func=AF.Square,
                                         accum_out=st64[:, pcol + 2:pcol + 3])
            for h in range(2):
                hs = slice(h * 512, (h + 1) * 512)
                ps_d = nrm_psum.tile([128, 512], F32, name=f"psd{ph}_{h}",
                                     tag="nrm")
                for k in range(KC):
                    prod = sq_pool.tile([128, 512], F32,
                                        name=f"pr{ph}_{k}_{h}", tag="sq")
                    nc.vector.tensor_mul(prod, xT[ph][k][:, hs],
                                         phat[ph][k][:, hs])
                    nc.tensor.matmul(ps_d, ones_t, prod,
                                     start=(k == 0), stop=(k == KC - 1))
                # ps_d already holds d~ = x . p-hat (phat is normalized);
                # the loss term additionally scales by 1/||x||
                lvec = scr_pool.tile([128, 512], F32, name=f"lv{ph}_{h}",
                                     tag="scr")
                nc.vector.tensor_mul(lvec, ps_d, invb_x[ph][:, hs])
                lcol = LCOL + ph * 2 + h
                nc.vector.tensor_reduce(comb[:, lcol:lcol + 1], lvec,
                                        axis=AX.X, op=ALU.add)
                for m in range(4):
                    mb = h * 4 + m
                    dscr = scr_pool.tile([128, 128], F32,
                                         name=f"ds{ph}_{mb}", tag="scr")
                    nc.vector.tensor_mul(dscr, ps_d[:, m * 128:(m + 1) * 128],
                                         i_t)
                    nc.vector.tensor_reduce(dT[ph][:, mb:mb + 1], dscr,
                                            axis=AX.X, op=ALU.add)
                nc.vector.tensor_scalar_mul(
                    negdT[ph][:, h * 4:h * 4 + 4], dT[ph][:, h * 4:h * 4 + 4],
                    -1.0)

        def main_col(ph, nt):
            r, cb = nt // 2, (nt % 2) * 512
            g = []
            for k in range(KC):
                gk = gpool.tile([128, NTILE], QDT, name=f"g{ph}_{nt}_{k}",
                                tag=f"g{k}")
                nc.sync.dma_start(
                    out=gk, in_=gath[ph][C * r + k * 128:C * r + (k + 1) * 128,
                                         cb:cb + NTILE])
                g.append(gk)
            for mb in range(MB):
                ps = mm_psum.tile([128, NTILE], F32, name=f"ps{ph}_{mb}_{nt}",
                                  tag="mm")
                for k in range(KC):
                    nc.tensor.matmul(ps, xT[ph][k][:, mb * 128:(mb + 1) * 128],
                                     g[k], start=(k == 0), stop=(k == KC - 1))
                if nt >= NT - N_ACT:
                    ascr = scr_pool.tile([128, NTILE], F32,
                                         name=f"a{ph}_{nt}_{mb}", tag="ascr")
                    nc.scalar.activation(
                        out=ascr, in_=ps, func=AF.Sign,
                        bias=negdT[ph][:, mb:mb + 1], scale=1.0,
                        accum_out=sgns[ph][:, mb, nt:nt + 1])
                else:
                    cscr = scr_pool.tile([128, NTILE], F32,
                                         name=f"c{ph}_{nt}_{mb}", tag="cscr")
                    nc.vector.tensor_scalar(
                        out=cscr, in0=ps, scalar1=dT[ph][:, mb:mb + 1],
                        scalar2=0.0, op0=ALU.is_gt, op1=ALU.add,
                        accum_out=cnts[ph][:, mb, nt:nt + 1])

        def reduce_slots(ph):
            cnt_t = scr_pool.tile([128, MB], F32, name=f"cntt{ph}", tag="mtc")
            sgn_t = scr_pool.tile([128, MB], F32, name=f"sgnt{ph}", tag="mts")
            pos = scr_pool.tile([128, MB], F32, name=f"pos{ph}", tag="mtp")
            for mb in range(MB):
                nc.vector.tensor_reduce(cnt_t[:, mb:mb + 1],
                                        cnts[ph][:, mb, :], axis=AX.X,
                                        op=ALU.add)
                nc.vector.tensor_reduce(sgn_t[:, mb:mb + 1],
                                        sgns[ph][:, mb, :], axis=AX.X,
                                        op=ALU.add)
            # pos = cnt + (sgn + C_ACT)/2, per local row
            nc.vector.tensor_scalar(out=pos, in0=sgn_t, scalar1=0.5,
                                    scalar2=C_ACT / 2.0, op0=ALU.mult,
                                    op1=ALU.add)
            nc.vector.tensor_add(pos, pos, cnt_t)
            base = MCOL + ph * 4
            for i, thr in enumerate((1.0, 5.0, 10.0)):
                mscr = scr_pool.tile([128, MB], F32, name=f"m{ph}_{i}",
                                     tag="mt2")
                nc.vector.tensor_scalar(
                    out=mscr, in0=pos, scalar1=thr, scalar2=0.0,
                    op0=ALU.is_lt, op1=ALU.add,
                    accum_out=comb[:, base + i:base + i + 1])
            nc.vector.tensor_reduce(comb[:, base + 3:base + 4], pos,
                                    axis=AX.X, op=ALU.add)

        # ---- emission order: fire both collectives as early as possible,
        # overlap them with the x-side local work, then stream the gathered
        # columns through the matmul+count pipeline ----
        for ph in range(2):
            for k in range(KC):
                nc.sync.dma_start(out=xT[ph][k],
                                  in_=xT_d[ph][k * 128:(k + 1) * 128, :])
        stage1(0)
        stage1(1)
        stage2(0)
        stage2(1)
        for nt in range(NT):
            main_col(0, nt)
        reduce_slots(0)
        for nt in range(NT):
            main_col(1, nt)
        reduce_slots(1)

        # pack st64 -> comb[0:32]: dst s*16 + t*4 + k <- sum over halves
        for t_ in range(4):
            for k in range(KC):
                for s_ in range(2):
                    srcc = t_ * 16 + k * 4 + s_ * 2
                    dst = s_ * 16 + t_ * 4 + k
                    nc.vector.tensor_reduce(comb[:, dst:dst + 1],
                                            st64[:, srcc:srcc + 2],
                                            axis=AX.X, op=ALU.add)

        ar_in = dram.tile([128, CCOLS], F32, name="ar_in")
        ar_out = dram.tile([128, CCOLS], F32, name="ar_out",
                           addr_space="Shared")
        nc.sync.dma_start(out=ar_in, in_=comb)
        nc.gpsimd.collective_compute(
            "AllReduce", ALU.add,
            replica_groups=[list(range(NCORES))],
            ins=[ar_in.opt()],
            outs=[ar_out.opt()])
        ged = persist.tile([128, CCOLS], F32, name="ged")
        nc.sync.dma_start(out=ged, in_=ar_out)

        # stats: per-feature var -> std -> mean over features
        stdsum = persist.tile([128, 4], F32, name="stdsum")
        for t_ in range(4):
            ssum = ged[:, t_ * 4:t_ * 4 + 4]
            ssq = ged[:, 16 + t_ * 4:16 + t_ * 4 + 4]
            v1 = scr_pool.tile([128, 4], F32, name=f"v1_{t_}", tag="fin")
            nc.vector.tensor_mul(v1, ssum, ssum)
            nc.vector.tensor_scalar_mul(v1, v1, 1.0 / N)
            nc.vector.tensor_sub(v1, ssq, v1)
            nc.vector.tensor_scalar_mul(v1, v1, 1.0 / (N - 1))
            nc.vector.tensor_scalar_max(v1, v1, 0.0)
            nc.scalar.sqrt(v1, v1)
            nc.vector.tensor_reduce(stdsum[:, t_:t_ + 1], v1, axis=AX.X,
                                    op=ALU.add)
        ps_f = nrm_psum.tile([128, 4], F32, name="ps_f", tag="nrm")
        nc.tensor.matmul(ps_f, ones_t, stdsum, start=True, stop=True)
        nc.vector.tensor_scalar_mul(fin[:, 1:5], ps_f, 1.0 / C)

        # retrieval metrics: partition-reduce partials, scale by 1/N
        ps_m = nrm_psum.tile([128, 8], F32, name="ps_m", tag="nrm")
        nc.tensor.matmul(ps_m, ones_t, ged[:, MCOL:MCOL + 8],
                         start=True, stop=True)
        nc.vector.tensor_scalar_mul(fin[:, 5:13], ps_m, 1.0 / N)

        # neg_sim = -0.5/N * sum of the 4 loss partials
        nc.vector.tensor_reduce(fin[:, 0:1], ged[:, LCOL:LCOL + 4],
                                axis=AX.X, op=ALU.add)
        nc.vector.tensor_scalar_mul(fin[:, 0:1], fin[:, 0:1], -0.5 / N)
        nc.vector.memset(fin[:, 13:16], 0.0)

        nc.sync.dma_start(out=o_fin_d, in_=fin[0:1, :])

    nc.compile()
    return nc


def _get_runner():
    """Build (once) a jitted 8-core SPMD executor for the Bass program.

    Mirrors bass2jax.run_bass_via_pjrt's multi-core branch, but keeps the
    jitted function and pre-staged device inputs so repeated calls skip
    retracing/recompiling, and so transfer vs execute can be timed apart.
    """
    if "runner" in _CACHE:
        return _CACHE["runner"]

    import jax
    from jax.experimental.shard_map import shard_map
    from jax.sharding import Mesh, PartitionSpec, NamedSharding
    from concourse import mybir as _mybir
    from concourse.bass2jax import (_bass_exec_p, install_neuronx_cc_hook,
                                    partition_id_tensor)

    nc = _CACHE["nc"]
    install_neuronx_cc_hook()

    partition_name = (nc.partition_id_tensor.name
                      if nc.partition_id_tensor else None)
    in_names, out_names, out_avals = [], [], []
    zero_outs = []
    for alloc in nc.m.functions[0].allocations:
        if not isinstance(alloc, _mybir.MemoryLocationSet):
            continue
        name = alloc.memorylocations[0].name
        if alloc.kind == "ExternalInput":
            if name != partition_name:
                in_names.append(name)
        elif alloc.kind == "ExternalOutput":
            out_names.append(name)
            shape = tuple(alloc.tensor_shape)
            dtype = _mybir.dt.np(alloc.dtype)
            out_avals.append(jax.core.ShapedArray(shape, dtype))
            zero_outs.append(np.zeros(shape, dtype))
    n_params = len(in_names)
    all_in_names = in_names + out_names
    if partition_name is not None:
        all_in_names = all_in_names + [partition_name]

    def _body(*args):
        operands = list(args)
        if partition_name is not None:
            operands.append(partition_id_tensor())
        outs = _bass_exec_p.bind(
            *operands,
            out_avals=tuple(out_avals),
            in_names=tuple(all_in_names),
            out_names=tuple(out_names),
            lowering_input_output_aliases=(),
            sim_require_finite=True,
            sim_require_nnan=True,
            nc=nc,
        )
        return tuple(outs)

    devices = jax.devices()[:NCORES]
    mesh = Mesh(np.asarray(devices), ("core",))
    spec = NamedSharding(mesh, PartitionSpec("core"))
    donate = tuple(range(n_params, n_params + len(out_names)))
    sharded = jax.jit(
        shard_map(_body, mesh=mesh,
                  in_specs=(PartitionSpec("core"),) * (n_params + len(out_names)),
                  out_specs=(PartitionSpec("core"),) * len(out_names),
                  check_rep=False),
        donate_argnums=donate, keep_unused=True)

    def run(in_maps):
        t0 = time.monotonic()
        concat_in = [
            np.concatenate([in_maps[c][name] for c in range(NCORES)], axis=0)
            for name in in_names
        ]
        dev_in = [jax.device_put(a, spec) for a in concat_in]
        dev_zero = [jax.device_put(
            np.zeros((NCORES * z.shape[0], *z.shape[1:]), z.dtype), spec)
            for z in zero_outs]
        for a in dev_in + dev_zero:
            a.block_until_ready()
        t1 = time.monotonic()
        out_arrs = sharded(*dev_in, *dev_zero)
        out_np = [np.asarray(a) for a in out_arrs]
        t2 = time.monotonic()
        TIMES.update(transfer_s=t1 - t0, execute_s=t2 - t1)
        return [
            {name: out_np[i].reshape(NCORES, *out_avals[i].shape)[c]
             for i, name in enumerate(out_names)}
            for c in range(NCORES)
        ]

    _CACHE["runner"] = run
    return run


def kernel(v_feat, t_feat, p_v, p_t):
    import ml_dtypes
    if "nc" not in _CACHE:
        _CACHE["nc"] = _build_program()

    t0 = time.monotonic()
    qdt = ml_dtypes.float8_e4m3
    v = np.asarray(v_feat, dtype=np.float32).astype(qdt)
    t = np.asarray(t_feat, dtype=np.float32).astype(qdt)
    pv = np.asarray(p_v, dtype=np.float32).astype(qdt)
    pt = np.asarray(p_t, dtype=np.float32).astype(qdt)

    in_maps = []
    for k in range(NCORES):
        sl = slice(k * S, (k + 1) * S)
        in_maps.append({
            "xin": np.concatenate([v[sl].T, t[sl].T, pt[sl].T, pv[sl].T],
                                  axis=0),
        })
    TIMES["prep_s"] = time.monotonic() - t0

    results = _get_runner()(in_maps)

    # the device finalizes everything; all cores return the identical [1,16]
    fin = results[0]["o_fin"][0]
    return np.asarray(fin[0:13], dtype=np.float32)
